# revision 2
# baseline (speedup 1.0000x reference)
"""Trainium2 Bass kernel for a 2-layer LSTM (H=64) + FC head.

Problem: x [4096, 168, 19] f32 -> out [4096] f32
  h1 = LSTM0(x); h2 = LSTM1(h1); out = h2[:, -1, :] @ Wfc.T + bfc

Data-parallel over 8 NeuronCores (512 batch rows each). Per core the
batch is split into CH=3 independent chains whose serial recurrences
interleave on the engines. Layer 0 at time w and layer 1 at time w-1
share one "wave" so every op uses all 128 partitions (p0:64 = layer0,
p64:128 = layer1).

All-tanh formulation: the sigmoid gates' weight rows are pre-halved on
the host so sigma(z) = tanh(z/2)*0.5+0.5; one ACT instruction then
applies tanh over the whole 4-bank z tile [G|I|F|O], and the sigmoid
affine runs on DVE (tensor_scalar, bf16 4x mode).

Software pipelining: tanh(c') and h = sigma_o * tanh(c') for wave w are
emitted at the TOP of wave w+1, so the ACT stream per wave is
[stc_c, T1_c] per chain and the c'->tanh->h->matmul round trip overlaps
the other chains' gate work.

Per wave per chain: PE: 8 matmuls (fp32r, N=CB); ACT: stc tanh [128,CB]
+ T1 tanh [128,4CB]; DVE: sigma ts [128,3CB] bf16 + u mul bf16; Pool:
h mul, v mul, c' add.
"""

import numpy as np

HIDDEN = 64
INPUT = 19
B = 4096
T = 168
NCORES = 8
BL = B // NCORES   # 512 per core
CBS = (128, 224, 160)
CH = len(CBS)
COFF = (0, 128, 352)
H4 = 4 * HIDDEN    # 256

# torch gate order rows: i(0:64) f(64:128) g(128:192) o(192:256)
# our bank (column-block) order: G, I, F, O
GATE_PERM = np.concatenate([
    np.arange(128, 192),  # g
    np.arange(0, 64),     # i
    np.arange(64, 128),   # f
    np.arange(192, 256),  # o
])
# tanh(z/2) trick for sigmoid gates: halve I, F, O rows (banks 1..3)
BANK_SCALE = np.array([1.0, 0.5, 0.5, 0.5], np.float32)


def build_nc(steps=T):
    import concourse.bacc as bacc
    import concourse.tile as tile
    from concourse import mybir

    F32 = mybir.dt.float32
    BF16 = mybir.dt.bfloat16
    MMD = BF16  # matmul operand dtype (1 cycle/row at any N)
    AF = mybir.ActivationFunctionType
    OP = mybir.AluOpType

    nc = bacc.Bacc("TRN2", target_bir_lowering=False, debug=False,
                   num_devices=NCORES)

    xT = nc.dram_tensor("xT", [T, INPUT + 1, BL], MMD, kind="ExternalInput")
    w0x_d = nc.dram_tensor("w0x", [INPUT + 1, 512], MMD, kind="ExternalInput")
    whbig_d = nc.dram_tensor("whbig", [128, 512], MMD, kind="ExternalInput")
    wfc_d = nc.dram_tensor("wfc", [128, 1], MMD, kind="ExternalInput")
    zeros_d = nc.dram_tensor("zeros", [128, max(CBS)], MMD,
                             kind="ExternalInput")
    out = nc.dram_tensor("out", [1, BL], F32, kind="ExternalOutput")

    with tile.TileContext(nc) as tc:
        with (
            tc.tile_pool(name="const", bufs=1) as const,
            tc.tile_pool(name="state", bufs=1) as state,
            tc.tile_pool(name="work", bufs=3) as work,
            tc.tile_pool(name="xin", bufs=5) as xin,
            tc.tile_pool(name="zpool", bufs=CH, space="PSUM") as zpool,
        ):
            w0x = const.tile([INPUT + 1, 4, 128], MMD, tag="w0x", name="w0x")
            whbig = const.tile([128, 4, 128], MMD, tag="wh", name="whbig")
            wfc = const.tile([128, 1], MMD, tag="wfc", name="wfc")
            nc.sync.dma_start(w0x, w0x_d[:])
            nc.sync.dma_start(whbig, whbig_d[:])
            nc.sync.dma_start(wfc, wfc_d[:])

            # per-chain state: C = cell (f32), hm = [h0; h1] (matmul rhs)
            C = [[state.tile([128, CBS[c]], F32, tag=f"C{c}{p}",
                             name=f"C{c}{p}") for p in (0, 1)]
                 for c in range(CH)]
            hm = [[state.tile([128, CBS[c]], MMD, tag=f"hm{c}{p}",
                              name=f"hm{c}{p}") for p in (0, 1)]
                  for c in range(CH)]
            for c in range(CH):
                nc.vector.memset(C[c][0], 0.0)
                nc.sync.dma_start(hm[c][0], zeros_d[:, 0:CBS[c]])

            nwaves = steps + 1
            so_prev = [None] * CH
            stc_bufs = [None] * CH

            def stage2(c, cur, partial=False):
                # tanh(c'@w-1) and h@w-1 = sigma_o * tanh(c'); writes hm[cur]
                CB = CBS[c]
                stc = work.tile([128, CB], BF16, tag=f"stc{c}",
                                name=f"stc{c}")
                lo = 64 if partial else 0
                nc.scalar.activation(stc[lo:128], C[c][cur][lo:128], AF.Tanh)
                nc.gpsimd.tensor_mul(hm[c][cur][lo:128],
                                     so_prev[c][lo:128], stc[lo:128])

            def stage1(c, w, cur, nxt):
                CB = CBS[c]
                cs = slice(COFF[c], COFF[c] + CB)
                xt = xin.tile([INPUT + 1, CB], MMD, tag=f"x{c}", name=f"x{c}")
                nc.sync.dma_start(xt, xT[w % T, :, cs])

                # pad bank slots to 256-f32 alignment so each matmul
                # output sits inside one 2KB PSUM bank
                z = zpool.tile([128, 4, CB], F32, tag="z", name=f"z{c}",
                               padded_shape=[128, 4, 256])
                # per gate-bank: mm1 = x-part + biases (K=20, ones-row
                # trick carries b0|b1); mm2 adds both layers' h-parts
                # (K=128) from hm = [h0; h1].
                for b in range(4):
                    nc.tensor.matmul(z[:, b, :], w0x[:, b, :], xt[:],
                                     start=True, stop=False,
                                     skip_group_check=True)
                    nc.tensor.matmul(z[:, b, :], whbig[:, b, :],
                                     hm[c][cur][:], start=False, stop=True,
                                     skip_group_check=True)

                t = work.tile([128, 4, CB], BF16, tag=f"t{c}", name=f"t{c}")
                nc.scalar.activation(t[:, :, :], z[:, :, :], AF.Tanh)
                sig = work.tile([128, 3, CB], BF16, tag=f"sig{c}",
                                name=f"sig{c}")
                u = work.tile([128, CB], BF16, tag=f"u{c}", name=f"u{c}")
                v = work.tile([128, CB], F32, tag=f"v{c}", name=f"v{c}")
                # column-halved c'-path: the second half of ts/u/v/c'
                # pipelines behind the first, shortening c' readiness
                hh = CB // 2
                for s in (slice(0, hh), slice(hh, CB)):
                    nc.vector.tensor_scalar(sig[:, :, s], t[:, 1:4, s],
                                            0.5, 0.5, OP.mult, OP.add)
                    nc.vector.tensor_mul(u[:, s], sig[:, 0, s], t[:, 0, s])
                    nc.gpsimd.tensor_mul(v[:, s], sig[:, 1, s],
                                         C[c][cur][:, s])
                    nc.gpsimd.tensor_add(C[c][nxt][:, s], u[:, s], v[:, s])
                so_prev[c] = sig[:, 2, :]

            for w in range(nwaves):
                cur, nxt = w % 2, (w + 1) % 2
                if w > 0:
                    for c in range(CH):
                        stage2(c, cur)
                for c in range(CH):
                    stage1(c, w, cur, nxt)
                if w == 0:
                    # wave 0's layer-1 half ran on garbage; zero its cell
                    # state; h@0 layer-1 = sigma_o*tanh(0) = 0 follows.
                    for c in range(CH):
                        nc.gpsimd.memset(C[c][nxt][64:128], 0.0)

            # --- tail: flush h@last (only layer-1 half matters) + FC ---
            cur_t = nwaves % 2
            o_sb = work.tile([1, BL], F32, tag="osb", name="o_sb")
            for c in range(CH):
                stage2(c, cur_t, partial=True)
            for c in range(CH):
                pfc = zpool.tile([1, CBS[c]], F32, tag="z", name=f"pfc{c}")
                nc.tensor.matmul(pfc, wfc, hm[c][cur_t][:],
                                 start=True, stop=True)
                nc.scalar.activation(
                    o_sb[:, COFF[c]:COFF[c] + CBS[c]], pfc, AF.Copy)
            nc.sync.dma_start(out[:], o_sb)

    nc.compile()
    return nc


def make_in_maps(x, Wih0, Whh0, bih0, bhh0, Wih1, Whh1, bih1, bhh1, Wfc, bfc):
    """Shard + pre-transpose/concat inputs for the 8 cores."""
    p = GATE_PERM
    s = np.repeat(BANK_SCALE, HIDDEN)  # [256] per permuted gate row
    b0 = (bih0 + bhh0)[p].astype(np.float32) * s
    b1 = (bih1 + bhh1)[p].astype(np.float32) * s
    W0i = Wih0[p] * s[:, None]
    W0h = Whh0[p] * s[:, None]
    W1i = Wih1[p] * s[:, None]
    W1h = Whh1[p] * s[:, None]
    # w0x [20, 4, 128]: rows = [x features (19); ones]. Left cols =
    # [Wih0; b0] per gate, right cols = b1 on the ones row.
    # whbig [128, 4, 128]: left cols = [Whh0; 0], right cols =
    # [Wih1; Whh1] -- one K=128 matmul vs hm covers both layers.
    w0x = np.zeros((INPUT + 1, 4, 128), np.float32)
    whbig = np.zeros((128, 4, 128), np.float32)
    for b in range(4):
        w0x[0:INPUT, b, 0:64] = W0i.T[:, b * 64:(b + 1) * 64]
        w0x[INPUT, b, 0:64] = b0[b * 64:(b + 1) * 64]
        w0x[INPUT, b, 64:128] = b1[b * 64:(b + 1) * 64]
        whbig[0:64, b, 0:64] = W0h.T[:, b * 64:(b + 1) * 64]
        whbig[0:64, b, 64:128] = W1i.T[:, b * 64:(b + 1) * 64]
        whbig[64:128, b, 64:128] = W1h.T[:, b * 64:(b + 1) * 64]
    wfcbig = np.zeros((128, 1), np.float32)
    wfcbig[64:128, 0] = Wfc.reshape(HIDDEN)
    import ml_dtypes
    bf16 = ml_dtypes.bfloat16
    base = {
        "w0x": np.ascontiguousarray(w0x.reshape(INPUT + 1, 512)).astype(bf16),
        "whbig": np.ascontiguousarray(whbig.reshape(128, 512)).astype(bf16),
        "wfc": wfcbig.astype(bf16),
        "zeros": np.zeros((128, max(CBS)), bf16),
    }
    xs = x.reshape(NCORES, BL, T, INPUT)
    in_maps = []
    for c in range(NCORES):
        m = dict(base)
        xt = np.empty((T, INPUT + 1, BL), bf16)
        xt[:, 0:INPUT, :] = xs[c].transpose(1, 2, 0).astype(bf16)
        xt[:, INPUT, :] = 1.0
        m["xT"] = xt
        in_maps.append(m)
    return in_maps


_CACHED_NC = None


def kernel(**inputs):
    global _CACHED_NC
    from concourse.bass_utils import run_bass_kernel_spmd

    if _CACHED_NC is None:
        _CACHED_NC = build_nc()
    nc = _CACHED_NC
    in_maps = make_in_maps(**inputs)
    res = run_bass_kernel_spmd(nc, in_maps, list(range(NCORES)))
    outs = [res.results[c]["out"].reshape(BL) for c in range(NCORES)]
    return np.concatenate(outs) + np.float32(inputs["bfc"][0])


# revision 3
# speedup vs baseline: 1.0124x; 1.0124x over previous
"""Trainium2 Bass kernel for a 2-layer LSTM (H=64) + FC head.

Problem: x [4096, 168, 19] f32 -> out [4096] f32
  h1 = LSTM0(x); h2 = LSTM1(h1); out = h2[:, -1, :] @ Wfc.T + bfc

Data-parallel over 8 NeuronCores (512 batch rows each). Per core the
batch is split into CH=3 independent chains whose serial recurrences
interleave on the engines. Layer 0 at time w and layer 1 at time w-1
share one "wave" so every op uses all 128 partitions (p0:64 = layer0,
p64:128 = layer1).

All-tanh formulation: the sigmoid gates' weight rows are pre-halved on
the host so sigma(z) = tanh(z/2)*0.5+0.5; one ACT instruction then
applies tanh over the whole 4-bank z tile [G|I|F|O], and the sigmoid
affine runs on DVE (tensor_scalar, bf16 4x mode).

Software pipelining: tanh(c') and h = sigma_o * tanh(c') for wave w are
emitted at the TOP of wave w+1, so the ACT stream per wave is
[stc_c, T1_c] per chain and the c'->tanh->h->matmul round trip overlaps
the other chains' gate work.

Per wave per chain: PE: 8 matmuls (fp32r, N=CB); ACT: stc tanh [128,CB]
+ T1 tanh [128,4CB]; DVE: sigma ts [128,3CB] bf16 + u mul bf16; Pool:
h mul, v mul, c' add.
"""

import numpy as np

HIDDEN = 64
INPUT = 19
B = 4096
T = 168
NCORES = 8
BL = B // NCORES   # 512 per core
CBS = (128, 232, 152)
CH = len(CBS)
COFF = (0, 128, 360)
H4 = 4 * HIDDEN    # 256

# torch gate order rows: i(0:64) f(64:128) g(128:192) o(192:256)
# our bank (column-block) order: G, I, F, O
GATE_PERM = np.concatenate([
    np.arange(128, 192),  # g
    np.arange(0, 64),     # i
    np.arange(64, 128),   # f
    np.arange(192, 256),  # o
])
# tanh(z/2) trick for sigmoid gates: halve I, F, O rows (banks 1..3)
BANK_SCALE = np.array([1.0, 0.5, 0.5, 0.5], np.float32)


def build_nc(steps=T):
    import concourse.bacc as bacc
    import concourse.tile as tile
    from concourse import mybir

    F32 = mybir.dt.float32
    BF16 = mybir.dt.bfloat16
    MMD = BF16  # matmul operand dtype (1 cycle/row at any N)
    AF = mybir.ActivationFunctionType
    OP = mybir.AluOpType

    nc = bacc.Bacc("TRN2", target_bir_lowering=False, debug=False,
                   num_devices=NCORES)

    xT = nc.dram_tensor("xT", [T, INPUT + 1, BL], MMD, kind="ExternalInput")
    w0x_d = nc.dram_tensor("w0x", [INPUT + 1, 512], MMD, kind="ExternalInput")
    whbig_d = nc.dram_tensor("whbig", [128, 512], MMD, kind="ExternalInput")
    wfc_d = nc.dram_tensor("wfc", [128, 1], MMD, kind="ExternalInput")
    zeros_d = nc.dram_tensor("zeros", [128, max(CBS)], MMD,
                             kind="ExternalInput")
    out = nc.dram_tensor("out", [1, BL], F32, kind="ExternalOutput")

    with tile.TileContext(nc) as tc:
        with (
            tc.tile_pool(name="const", bufs=1) as const,
            tc.tile_pool(name="state", bufs=1) as state,
            tc.tile_pool(name="work", bufs=3) as work,
            tc.tile_pool(name="xin", bufs=5) as xin,
            tc.tile_pool(name="zpool", bufs=CH, space="PSUM") as zpool,
        ):
            w0x = const.tile([INPUT + 1, 4, 128], MMD, tag="w0x", name="w0x")
            whbig = const.tile([128, 4, 128], MMD, tag="wh", name="whbig")
            wfc = const.tile([128, 1], MMD, tag="wfc", name="wfc")
            nc.sync.dma_start(w0x, w0x_d[:])
            nc.sync.dma_start(whbig, whbig_d[:])
            nc.sync.dma_start(wfc, wfc_d[:])

            # per-chain state: C = cell (f32), hm = [h0; h1] (matmul rhs)
            C = [[state.tile([128, CBS[c]], F32, tag=f"C{c}{p}",
                             name=f"C{c}{p}") for p in (0, 1)]
                 for c in range(CH)]
            hm = [[state.tile([128, CBS[c]], MMD, tag=f"hm{c}{p}",
                              name=f"hm{c}{p}") for p in (0, 1)]
                  for c in range(CH)]
            for c in range(CH):
                nc.vector.memset(C[c][0], 0.0)
                nc.sync.dma_start(hm[c][0], zeros_d[:, 0:CBS[c]])

            nwaves = steps + 1
            so_prev = [None] * CH
            stc_bufs = [None] * CH

            def stage2(c, cur, partial=False):
                # tanh(c'@w-1) and h@w-1 = sigma_o * tanh(c'); writes hm[cur]
                CB = CBS[c]
                stc = work.tile([128, CB], BF16, tag=f"stc{c}",
                                name=f"stc{c}")
                lo = 64 if partial else 0
                nc.scalar.activation(stc[lo:128], C[c][cur][lo:128], AF.Tanh)
                nc.gpsimd.tensor_mul(hm[c][cur][lo:128],
                                     so_prev[c][lo:128], stc[lo:128])

            def stage1(c, w, cur, nxt):
                CB = CBS[c]
                cs = slice(COFF[c], COFF[c] + CB)
                xt = xin.tile([INPUT + 1, CB], MMD, tag=f"x{c}", name=f"x{c}")
                nc.sync.dma_start(xt, xT[w % T, :, cs])

                # pad bank slots to 256-f32 alignment so each matmul
                # output sits inside one 2KB PSUM bank
                z = zpool.tile([128, 4, CB], F32, tag="z", name=f"z{c}",
                               padded_shape=[128, 4, 256])
                # per gate-bank: mm1 = x-part + biases (K=20, ones-row
                # trick carries b0|b1); mm2 adds both layers' h-parts
                # (K=128) from hm = [h0; h1].
                for b in range(4):
                    nc.tensor.matmul(z[:, b, :], w0x[:, b, :], xt[:],
                                     start=True, stop=False,
                                     skip_group_check=True)
                    nc.tensor.matmul(z[:, b, :], whbig[:, b, :],
                                     hm[c][cur][:], start=False, stop=True,
                                     skip_group_check=True)

                t = work.tile([128, 4, CB], BF16, tag=f"t{c}", name=f"t{c}")
                nc.scalar.activation(t[:, :, :], z[:, :, :], AF.Tanh)
                sig = work.tile([128, 3, CB], BF16, tag=f"sig{c}",
                                name=f"sig{c}")
                u = work.tile([128, CB], BF16, tag=f"u{c}", name=f"u{c}")
                v = work.tile([128, CB], F32, tag=f"v{c}", name=f"v{c}")
                # column-halved c'-path: the second half of ts/u/v/c'
                # pipelines behind the first, shortening c' readiness
                hh = CB // 2
                for s in (slice(0, hh), slice(hh, CB)):
                    nc.vector.tensor_scalar(sig[:, :, s], t[:, 1:4, s],
                                            0.5, 0.5, OP.mult, OP.add)
                    nc.vector.tensor_mul(u[:, s], sig[:, 0, s], t[:, 0, s])
                    nc.gpsimd.tensor_mul(v[:, s], sig[:, 1, s],
                                         C[c][cur][:, s])
                    nc.gpsimd.tensor_add(C[c][nxt][:, s], u[:, s], v[:, s])
                so_prev[c] = sig[:, 2, :]

            for w in range(nwaves):
                cur, nxt = w % 2, (w + 1) % 2
                if w > 0:
                    for c in range(CH):
                        stage2(c, cur)
                for c in range(CH):
                    stage1(c, w, cur, nxt)
                if w == 0:
                    # wave 0's layer-1 half ran on garbage; zero its cell
                    # state; h@0 layer-1 = sigma_o*tanh(0) = 0 follows.
                    for c in range(CH):
                        nc.gpsimd.memset(C[c][nxt][64:128], 0.0)

            # --- tail: flush h@last (only layer-1 half matters) + FC ---
            cur_t = nwaves % 2
            o_sb = work.tile([1, BL], F32, tag="osb", name="o_sb")
            for c in range(CH):
                stage2(c, cur_t, partial=True)
            for c in range(CH):
                pfc = zpool.tile([1, CBS[c]], F32, tag="z", name=f"pfc{c}")
                nc.tensor.matmul(pfc, wfc, hm[c][cur_t][:],
                                 start=True, stop=True)
                nc.scalar.activation(
                    o_sb[:, COFF[c]:COFF[c] + CBS[c]], pfc, AF.Copy)
            nc.sync.dma_start(out[:], o_sb)

    nc.compile()
    return nc


def make_in_maps(x, Wih0, Whh0, bih0, bhh0, Wih1, Whh1, bih1, bhh1, Wfc, bfc):
    """Shard + pre-transpose/concat inputs for the 8 cores."""
    p = GATE_PERM
    s = np.repeat(BANK_SCALE, HIDDEN)  # [256] per permuted gate row
    b0 = (bih0 + bhh0)[p].astype(np.float32) * s
    b1 = (bih1 + bhh1)[p].astype(np.float32) * s
    W0i = Wih0[p] * s[:, None]
    W0h = Whh0[p] * s[:, None]
    W1i = Wih1[p] * s[:, None]
    W1h = Whh1[p] * s[:, None]
    # w0x [20, 4, 128]: rows = [x features (19); ones]. Left cols =
    # [Wih0; b0] per gate, right cols = b1 on the ones row.
    # whbig [128, 4, 128]: left cols = [Whh0; 0], right cols =
    # [Wih1; Whh1] -- one K=128 matmul vs hm covers both layers.
    w0x = np.zeros((INPUT + 1, 4, 128), np.float32)
    whbig = np.zeros((128, 4, 128), np.float32)
    for b in range(4):
        w0x[0:INPUT, b, 0:64] = W0i.T[:, b * 64:(b + 1) * 64]
        w0x[INPUT, b, 0:64] = b0[b * 64:(b + 1) * 64]
        w0x[INPUT, b, 64:128] = b1[b * 64:(b + 1) * 64]
        whbig[0:64, b, 0:64] = W0h.T[:, b * 64:(b + 1) * 64]
        whbig[0:64, b, 64:128] = W1i.T[:, b * 64:(b + 1) * 64]
        whbig[64:128, b, 64:128] = W1h.T[:, b * 64:(b + 1) * 64]
    wfcbig = np.zeros((128, 1), np.float32)
    wfcbig[64:128, 0] = Wfc.reshape(HIDDEN)
    import ml_dtypes
    bf16 = ml_dtypes.bfloat16
    base = {
        "w0x": np.ascontiguousarray(w0x.reshape(INPUT + 1, 512)).astype(bf16),
        "whbig": np.ascontiguousarray(whbig.reshape(128, 512)).astype(bf16),
        "wfc": wfcbig.astype(bf16),
        "zeros": np.zeros((128, max(CBS)), bf16),
    }
    xs = x.reshape(NCORES, BL, T, INPUT)
    in_maps = []
    for c in range(NCORES):
        m = dict(base)
        xt = np.empty((T, INPUT + 1, BL), bf16)
        xt[:, 0:INPUT, :] = xs[c].transpose(1, 2, 0).astype(bf16)
        xt[:, INPUT, :] = 1.0
        m["xT"] = xt
        in_maps.append(m)
    return in_maps


_CACHED_NC = None


def kernel(**inputs):
    global _CACHED_NC
    from concourse.bass_utils import run_bass_kernel_spmd

    if _CACHED_NC is None:
        _CACHED_NC = build_nc()
    nc = _CACHED_NC
    in_maps = make_in_maps(**inputs)
    res = run_bass_kernel_spmd(nc, in_maps, list(range(NCORES)))
    outs = [res.results[c]["out"].reshape(BL) for c in range(NCORES)]
    return np.concatenate(outs) + np.float32(inputs["bfc"][0])


# revision 4
# speedup vs baseline: 1.0129x; 1.0005x over previous
"""Trainium2 Bass kernel for a 2-layer LSTM (H=64) + FC head.

Problem: x [4096, 168, 19] f32 -> out [4096] f32
  h1 = LSTM0(x); h2 = LSTM1(h1); out = h2[:, -1, :] @ Wfc.T + bfc

Data-parallel over 8 NeuronCores (512 batch rows each). Per core the
batch is split into CH=3 independent chains whose serial recurrences
interleave on the engines. Layer 0 at time w and layer 1 at time w-1
share one "wave" so every op uses all 128 partitions (p0:64 = layer0,
p64:128 = layer1).

All-tanh formulation: the sigmoid gates' weight rows are pre-halved on
the host so sigma(z) = tanh(z/2)*0.5+0.5; one ACT instruction then
applies tanh over the whole 4-bank z tile [G|I|F|O], and the sigmoid
affine runs on DVE (tensor_scalar, bf16 4x mode).

Software pipelining: tanh(c') and h = sigma_o * tanh(c') for wave w are
emitted at the TOP of wave w+1, so the ACT stream per wave is
[stc_c, T1_c] per chain and the c'->tanh->h->matmul round trip overlaps
the other chains' gate work.

Per wave per chain: PE: 8 bf16 matmuls (N=CB); ACT: stc tanh [128,CB]
+ T1 tanh [128,4CB]; DVE: sigma ts [128,3CB] bf16 4x + u mul bf16 2x;
Pool: h mul, v mul, c' add (no access bubble on Pool). The c'-path ops
are column-halved so the second half pipelines behind the first.
Chain sizes (128,232,152) tessellate the in-order ACT stream
[stc_A,stc_B,stc_C,T1_A,T1_B,T1_C] to ~49ns idle per 3293ns wave
(ACT busy floor 3244ns).
"""

import numpy as np

HIDDEN = 64
INPUT = 19
B = 4096
T = 168
NCORES = 8
BL = B // NCORES   # 512 per core
CBS = (128, 232, 152)
CH = len(CBS)
COFF = (0, 128, 360)
H4 = 4 * HIDDEN    # 256

# torch gate order rows: i(0:64) f(64:128) g(128:192) o(192:256)
# our bank (column-block) order: G, I, F, O
GATE_PERM = np.concatenate([
    np.arange(128, 192),  # g
    np.arange(0, 64),     # i
    np.arange(64, 128),   # f
    np.arange(192, 256),  # o
])
# tanh(z/2) trick for sigmoid gates: halve I, F, O rows (banks 1..3)
BANK_SCALE = np.array([1.0, 0.5, 0.5, 0.5], np.float32)


def build_nc(steps=T):
    import concourse.bacc as bacc
    import concourse.tile as tile
    from concourse import mybir

    F32 = mybir.dt.float32
    BF16 = mybir.dt.bfloat16
    MMD = BF16  # matmul operand dtype (1 cycle/row at any N)
    AF = mybir.ActivationFunctionType
    OP = mybir.AluOpType

    nc = bacc.Bacc("TRN2", target_bir_lowering=False, debug=False,
                   num_devices=NCORES)

    xT = nc.dram_tensor("xT", [T, INPUT + 1, BL], MMD, kind="ExternalInput")
    w0x_d = nc.dram_tensor("w0x", [INPUT + 1, 512], MMD, kind="ExternalInput")
    whbig_d = nc.dram_tensor("whbig", [128, 512], MMD, kind="ExternalInput")
    wfc_d = nc.dram_tensor("wfc", [128, 1], MMD, kind="ExternalInput")
    out = nc.dram_tensor("out", [1, BL], F32, kind="ExternalOutput")

    with tile.TileContext(nc) as tc:
        with (
            tc.tile_pool(name="const", bufs=1) as const,
            tc.tile_pool(name="state", bufs=1) as state,
            tc.tile_pool(name="work", bufs=3) as work,
            tc.tile_pool(name="xin", bufs=5) as xin,
            tc.tile_pool(name="zpool", bufs=CH, space="PSUM") as zpool,
        ):
            w0x = const.tile([INPUT + 1, 4, 128], MMD, tag="w0x", name="w0x")
            whbig = const.tile([128, 4, 128], MMD, tag="wh", name="whbig")
            wfc = const.tile([128, 1], MMD, tag="wfc", name="wfc")
            nc.sync.dma_start(w0x, w0x_d[:])
            nc.scalar.dma_start(whbig, whbig_d[:])
            nc.scalar.dma_start(wfc, wfc_d[:])

            # per-chain state: C = cell (f32), hm = [h0; h1] (matmul rhs)
            C = [[state.tile([128, CBS[c]], F32, tag=f"C{c}{p}",
                             name=f"C{c}{p}") for p in (0, 1)]
                 for c in range(CH)]
            hm = [[state.tile([128, CBS[c]], MMD, tag=f"hm{c}{p}",
                              name=f"hm{c}{p}") for p in (0, 1)]
                  for c in range(CH)]
            for c in range(CH):
                nc.vector.memset(C[c][0], 0.0)
                nc.gpsimd.memset(hm[c][0], 0.0)

            nwaves = steps + 1
            so_prev = [None] * CH
            stc_bufs = [None] * CH

            def stage2(c, cur, partial=False):
                # tanh(c'@w-1) and h@w-1 = sigma_o * tanh(c'); writes hm[cur]
                CB = CBS[c]
                stc = work.tile([128, CB], BF16, tag=f"stc{c}",
                                name=f"stc{c}")
                lo = 64 if partial else 0
                nc.scalar.activation(stc[lo:128], C[c][cur][lo:128], AF.Tanh)
                nc.gpsimd.tensor_mul(hm[c][cur][lo:128],
                                     so_prev[c][lo:128], stc[lo:128])

            def stage1(c, w, cur, nxt):
                CB = CBS[c]
                cs = slice(COFF[c], COFF[c] + CB)
                xt = xin.tile([INPUT + 1, CB], MMD, tag=f"x{c}", name=f"x{c}")
                nc.sync.dma_start(xt, xT[w % T, :, cs])

                # pad bank slots to 256-f32 alignment so each matmul
                # output sits inside one 2KB PSUM bank
                z = zpool.tile([128, 4, CB], F32, tag="z", name=f"z{c}",
                               padded_shape=[128, 4, 256])
                # per gate-bank: mm1 = x-part + biases (K=20, ones-row
                # trick carries b0|b1); mm2 adds both layers' h-parts
                # (K=128) from hm = [h0; h1].
                for b in range(4):
                    nc.tensor.matmul(z[:, b, :], w0x[:, b, :], xt[:],
                                     start=True, stop=False,
                                     skip_group_check=True)
                    nc.tensor.matmul(z[:, b, :], whbig[:, b, :],
                                     hm[c][cur][:], start=False, stop=True,
                                     skip_group_check=True)

                t = work.tile([128, 4, CB], BF16, tag=f"t{c}", name=f"t{c}")
                nc.scalar.activation(t[:, :, :], z[:, :, :], AF.Tanh)
                sig = work.tile([128, 3, CB], BF16, tag=f"sig{c}",
                                name=f"sig{c}")
                u = work.tile([128, CB], BF16, tag=f"u{c}", name=f"u{c}")
                v = work.tile([128, CB], F32, tag=f"v{c}", name=f"v{c}")
                # column-halved c'-path: the second half of ts/u/v/c'
                # pipelines behind the first, shortening c' readiness
                hh = CB // 2
                for s in (slice(0, hh), slice(hh, CB)):
                    nc.vector.tensor_scalar(sig[:, :, s], t[:, 1:4, s],
                                            0.5, 0.5, OP.mult, OP.add)
                    nc.vector.tensor_mul(u[:, s], sig[:, 0, s], t[:, 0, s])
                    nc.gpsimd.tensor_mul(v[:, s], sig[:, 1, s],
                                         C[c][cur][:, s])
                    nc.gpsimd.tensor_add(C[c][nxt][:, s], u[:, s], v[:, s])
                so_prev[c] = sig[:, 2, :]

            for w in range(nwaves):
                cur, nxt = w % 2, (w + 1) % 2
                if w > 0:
                    for c in range(CH):
                        stage2(c, cur)
                for c in range(CH):
                    stage1(c, w, cur, nxt)
                if w == 0:
                    # wave 0's layer-1 half ran on garbage; zero its cell
                    # state; h@0 layer-1 = sigma_o*tanh(0) = 0 follows.
                    for c in range(CH):
                        nc.gpsimd.memset(C[c][nxt][64:128], 0.0)

            # --- tail: flush h@last (only layer-1 half matters) + FC ---
            cur_t = nwaves % 2
            o_sb = work.tile([1, BL], F32, tag="osb", name="o_sb")
            for c in range(CH):
                stage2(c, cur_t, partial=True)
            for c in range(CH):
                pfc = zpool.tile([1, CBS[c]], F32, tag="z", name=f"pfc{c}")
                nc.tensor.matmul(pfc, wfc, hm[c][cur_t][:],
                                 start=True, stop=True)
                nc.scalar.activation(
                    o_sb[:, COFF[c]:COFF[c] + CBS[c]], pfc, AF.Copy)
            nc.sync.dma_start(out[:], o_sb)

    nc.compile()
    return nc


def make_in_maps(x, Wih0, Whh0, bih0, bhh0, Wih1, Whh1, bih1, bhh1, Wfc, bfc):
    """Shard + pre-transpose/concat inputs for the 8 cores."""
    p = GATE_PERM
    s = np.repeat(BANK_SCALE, HIDDEN)  # [256] per permuted gate row
    b0 = (bih0 + bhh0)[p].astype(np.float32) * s
    b1 = (bih1 + bhh1)[p].astype(np.float32) * s
    W0i = Wih0[p] * s[:, None]
    W0h = Whh0[p] * s[:, None]
    W1i = Wih1[p] * s[:, None]
    W1h = Whh1[p] * s[:, None]
    # w0x [20, 4, 128]: rows = [x features (19); ones]. Left cols =
    # [Wih0; b0] per gate, right cols = b1 on the ones row.
    # whbig [128, 4, 128]: left cols = [Whh0; 0], right cols =
    # [Wih1; Whh1] -- one K=128 matmul vs hm covers both layers.
    w0x = np.zeros((INPUT + 1, 4, 128), np.float32)
    whbig = np.zeros((128, 4, 128), np.float32)
    for b in range(4):
        w0x[0:INPUT, b, 0:64] = W0i.T[:, b * 64:(b + 1) * 64]
        w0x[INPUT, b, 0:64] = b0[b * 64:(b + 1) * 64]
        w0x[INPUT, b, 64:128] = b1[b * 64:(b + 1) * 64]
        whbig[0:64, b, 0:64] = W0h.T[:, b * 64:(b + 1) * 64]
        whbig[0:64, b, 64:128] = W1i.T[:, b * 64:(b + 1) * 64]
        whbig[64:128, b, 64:128] = W1h.T[:, b * 64:(b + 1) * 64]
    wfcbig = np.zeros((128, 1), np.float32)
    wfcbig[64:128, 0] = Wfc.reshape(HIDDEN)
    import ml_dtypes
    bf16 = ml_dtypes.bfloat16
    base = {
        "w0x": np.ascontiguousarray(w0x.reshape(INPUT + 1, 512)).astype(bf16),
        "whbig": np.ascontiguousarray(whbig.reshape(128, 512)).astype(bf16),
        "wfc": wfcbig.astype(bf16),
    }
    xs = x.reshape(NCORES, BL, T, INPUT)
    in_maps = []
    for c in range(NCORES):
        m = dict(base)
        xt = np.empty((T, INPUT + 1, BL), bf16)
        xt[:, 0:INPUT, :] = xs[c].transpose(1, 2, 0).astype(bf16)
        xt[:, INPUT, :] = 1.0
        m["xT"] = xt
        in_maps.append(m)
    return in_maps


_CACHED_NC = None


def kernel(**inputs):
    global _CACHED_NC
    from concourse.bass_utils import run_bass_kernel_spmd

    if _CACHED_NC is None:
        _CACHED_NC = build_nc()
    nc = _CACHED_NC
    in_maps = make_in_maps(**inputs)
    res = run_bass_kernel_spmd(nc, in_maps, list(range(NCORES)))
    outs = [res.results[c]["out"].reshape(BL) for c in range(NCORES)]
    return np.concatenate(outs) + np.float32(inputs["bfc"][0])


# revision 5
# speedup vs baseline: 1.0221x; 1.0091x over previous
"""Trainium2 Bass kernel for a 2-layer LSTM (H=64) + FC head.

Problem: x [4096, 168, 19] f32 -> out [4096] f32
  h1 = LSTM0(x); h2 = LSTM1(h1); out = h2[:, -1, :] @ Wfc.T + bfc

Data-parallel over 8 NeuronCores (512 batch rows each). Per core the
batch is split into CH=3 independent chains whose serial recurrences
interleave on the engines. Layer 0 at time w and layer 1 at time w-1
share one "wave" so every op uses all 128 partitions (p0:64 = layer0,
p64:128 = layer1).

All-tanh formulation: the sigmoid gates' weight rows are pre-halved on
the host so sigma(z) = tanh(z/2)*0.5+0.5; one ACT instruction then
applies tanh over the whole 4-bank z tile [G|I|F|O], and the sigmoid
affine runs on DVE (tensor_scalar, bf16 4x mode).

Software pipelining: tanh(c') and h = sigma_o * tanh(c') for wave w are
emitted at the TOP of wave w+1, so the ACT stream per wave is
[stc_c, T1_c] per chain and the c'->tanh->h->matmul round trip overlaps
the other chains' gate work.

Per wave per chain: PE: 8 bf16 matmuls (N=CB); ACT: stc tanh [128,CB]
+ T1 tanh [128,4CB]; DVE: sigma ts [128,3CB] bf16 4x + u mul bf16 2x;
Pool: h mul, v mul, c' add (no access bubble on Pool). The c'-path ops
are column-halved so the second half pipelines behind the first.
Chain sizes (128,232,152) tessellate the in-order ACT stream
[stc_A,stc_B,stc_C,T1_A,T1_B,T1_C] to ~49ns idle per 3293ns wave
(ACT busy floor 3244ns).
"""

import numpy as np

HIDDEN = 64
INPUT = 19
B = 4096
T = 168
NCORES = 8
BL = B // NCORES   # 512 per core
CBS = (128, 232, 152)
CH = len(CBS)
COFF = (0, 128, 360)
H4 = 4 * HIDDEN    # 256

# torch gate order rows: i(0:64) f(64:128) g(128:192) o(192:256)
# our bank (column-block) order: G, I, F, O
GATE_PERM = np.concatenate([
    np.arange(128, 192),  # g
    np.arange(0, 64),     # i
    np.arange(64, 128),   # f
    np.arange(192, 256),  # o
])
# tanh(z/2) trick for sigmoid gates: halve I, F, O rows (banks 1..3)
BANK_SCALE = np.array([1.0, 0.5, 0.5, 0.5], np.float32)


def build_nc(steps=T):
    import concourse.bacc as bacc
    import concourse.tile as tile
    from concourse import mybir

    F32 = mybir.dt.float32
    BF16 = mybir.dt.bfloat16
    MMD = BF16  # matmul operand dtype (1 cycle/row at any N)
    AF = mybir.ActivationFunctionType
    OP = mybir.AluOpType

    nc = bacc.Bacc("TRN2", target_bir_lowering=False, debug=False,
                   num_devices=NCORES)

    xT = nc.dram_tensor("xT", [T, INPUT + 1, BL], MMD, kind="ExternalInput")
    w0x_d = nc.dram_tensor("w0x", [INPUT + 1, 512], MMD, kind="ExternalInput")
    whbig_d = nc.dram_tensor("whbig", [128, 512], MMD, kind="ExternalInput")
    wfc_d = nc.dram_tensor("wfc", [128, 1], MMD, kind="ExternalInput")
    out = nc.dram_tensor("out", [1, BL], F32, kind="ExternalOutput")

    with tile.TileContext(nc) as tc:
        with (
            tc.tile_pool(name="const", bufs=1) as const,
            tc.tile_pool(name="state", bufs=1) as state,
            tc.tile_pool(name="work", bufs=3) as work,
            tc.tile_pool(name="xin", bufs=5) as xin,
            tc.tile_pool(name="zpool", bufs=CH, space="PSUM") as zpool,
        ):
            w0x = const.tile([INPUT + 1, 4, 128], MMD, tag="w0x", name="w0x")
            whbig = const.tile([128, 4, 128], MMD, tag="wh", name="whbig")
            wfc = const.tile([128, 1], MMD, tag="wfc", name="wfc")
            nc.sync.dma_start(w0x, w0x_d[:])
            nc.scalar.dma_start(whbig, whbig_d[:])
            nc.scalar.dma_start(wfc, wfc_d[:])

            # per-chain state: C = cell (f32), hm = [h0; h1] (matmul rhs)
            C = [[state.tile([128, CBS[c]], F32, tag=f"C{c}{p}",
                             name=f"C{c}{p}") for p in (0, 1)]
                 for c in range(CH)]
            hm = [[state.tile([128, CBS[c]], MMD, tag=f"hm{c}{p}",
                              name=f"hm{c}{p}") for p in (0, 1)]
                  for c in range(CH)]
            for c in range(CH):
                nc.vector.memset(C[c][0], 0.0)
                nc.gpsimd.memset(hm[c][0], 0.0)

            nwaves = steps + 1
            so_prev = [None] * CH
            stc_bufs = [None] * CH

            def stage2(c, cur, partial=False):
                # tanh(c'@w-1) and h@w-1 = sigma_o * tanh(c'); writes hm[cur]
                CB = CBS[c]
                stc = work.tile([128, CB], BF16, tag=f"stc{c}",
                                name=f"stc{c}")
                lo = 64 if partial else 0
                nc.scalar.activation(stc[lo:128], C[c][cur][lo:128], AF.Tanh)
                nc.gpsimd.tensor_mul(hm[c][cur][lo:128],
                                     so_prev[c][lo:128], stc[lo:128])

            def stage1(c, w, cur, nxt):
                CB = CBS[c]
                cs = slice(COFF[c], COFF[c] + CB)
                xt = xin.tile([INPUT + 1, CB], MMD, tag=f"x{c}", name=f"x{c}")
                nc.sync.dma_start(xt, xT[w % T, :, cs])

                # pad bank slots to 256-f32 alignment so each matmul
                # output sits inside one 2KB PSUM bank
                z = zpool.tile([128, 4, CB], F32, tag="z", name=f"z{c}",
                               padded_shape=[128, 4, 256])
                # per gate-bank: mm1 = x-part + biases (K=20, ones-row
                # trick carries b0|b1); mm2 adds both layers' h-parts
                # (K=128) from hm = [h0; h1].
                for b in range(4):
                    nc.tensor.matmul(z[:, b, :], w0x[:, b, :], xt[:],
                                     start=True, stop=False,
                                     skip_group_check=True)
                    nc.tensor.matmul(z[:, b, :], whbig[:, b, :],
                                     hm[c][cur][:], start=False, stop=True,
                                     skip_group_check=True)

                t = work.tile([128, 4, CB], BF16, tag=f"t{c}", name=f"t{c}")
                nc.scalar.activation(t[:, :, :], z[:, :, :], AF.Tanh)
                sig = work.tile([128, 3, CB], BF16, tag=f"sig{c}",
                                name=f"sig{c}")
                u = work.tile([128, CB], BF16, tag=f"u{c}", name=f"u{c}")
                v = work.tile([128, CB], F32, tag=f"v{c}", name=f"v{c}")
                # column-halved c'-path: the second half of ts/u/v/c'
                # pipelines behind the first, shortening c' readiness
                hh = CB // 2
                for s in (slice(0, hh), slice(hh, CB)):
                    nc.vector.tensor_scalar(sig[:, :, s], t[:, 1:4, s],
                                            0.5, 0.5, OP.mult, OP.add)
                    nc.vector.tensor_mul(u[:, s], sig[:, 0, s], t[:, 0, s])
                    nc.gpsimd.tensor_mul(v[:, s], sig[:, 1, s],
                                         C[c][cur][:, s])
                    nc.gpsimd.tensor_add(C[c][nxt][:, s], u[:, s], v[:, s])
                so_prev[c] = sig[:, 2, :]

            for w in range(nwaves):
                cur, nxt = w % 2, (w + 1) % 2
                if w > 0:
                    for c in range(CH):
                        stage2(c, cur)
                for c in range(CH):
                    stage1(c, w, cur, nxt)
                if w == 0:
                    # wave 0's layer-1 half ran on garbage; zero its cell
                    # state; h@0 layer-1 = sigma_o*tanh(0) = 0 follows.
                    for c in range(CH):
                        nc.gpsimd.memset(C[c][nxt][64:128], 0.0)

            # --- tail: flush h@last (only layer-1 half matters) + FC ---
            # pfc -> o_sb copies ride the idle DVE so they overlap the
            # tail stc stream on ACT; one DMA ships the assembled row.
            cur_t = nwaves % 2
            o_sb = work.tile([1, BL], F32, tag="osb", name="o_sb")
            for c in range(CH):
                stage2(c, cur_t, partial=True)
            for c in range(CH):
                pfc = zpool.tile([1, CBS[c]], F32, tag="z", name=f"pfc{c}")
                nc.tensor.matmul(pfc, wfc, hm[c][cur_t][:],
                                 start=True, stop=True)
                nc.vector.tensor_copy(
                    o_sb[:, COFF[c]:COFF[c] + CBS[c]], pfc)
            nc.sync.dma_start(out[:], o_sb)

    nc.compile()
    return nc


def make_in_maps(x, Wih0, Whh0, bih0, bhh0, Wih1, Whh1, bih1, bhh1, Wfc, bfc):
    """Shard + pre-transpose/concat inputs for the 8 cores."""
    p = GATE_PERM
    s = np.repeat(BANK_SCALE, HIDDEN)  # [256] per permuted gate row
    b0 = (bih0 + bhh0)[p].astype(np.float32) * s
    b1 = (bih1 + bhh1)[p].astype(np.float32) * s
    W0i = Wih0[p] * s[:, None]
    W0h = Whh0[p] * s[:, None]
    W1i = Wih1[p] * s[:, None]
    W1h = Whh1[p] * s[:, None]
    # w0x [20, 4, 128]: rows = [x features (19); ones]. Left cols =
    # [Wih0; b0] per gate, right cols = b1 on the ones row.
    # whbig [128, 4, 128]: left cols = [Whh0; 0], right cols =
    # [Wih1; Whh1] -- one K=128 matmul vs hm covers both layers.
    w0x = np.zeros((INPUT + 1, 4, 128), np.float32)
    whbig = np.zeros((128, 4, 128), np.float32)
    for b in range(4):
        w0x[0:INPUT, b, 0:64] = W0i.T[:, b * 64:(b + 1) * 64]
        w0x[INPUT, b, 0:64] = b0[b * 64:(b + 1) * 64]
        w0x[INPUT, b, 64:128] = b1[b * 64:(b + 1) * 64]
        whbig[0:64, b, 0:64] = W0h.T[:, b * 64:(b + 1) * 64]
        whbig[0:64, b, 64:128] = W1i.T[:, b * 64:(b + 1) * 64]
        whbig[64:128, b, 64:128] = W1h.T[:, b * 64:(b + 1) * 64]
    wfcbig = np.zeros((128, 1), np.float32)
    wfcbig[64:128, 0] = Wfc.reshape(HIDDEN)
    import ml_dtypes
    bf16 = ml_dtypes.bfloat16
    base = {
        "w0x": np.ascontiguousarray(w0x.reshape(INPUT + 1, 512)).astype(bf16),
        "whbig": np.ascontiguousarray(whbig.reshape(128, 512)).astype(bf16),
        "wfc": wfcbig.astype(bf16),
    }
    xs = x.reshape(NCORES, BL, T, INPUT)
    in_maps = []
    for c in range(NCORES):
        m = dict(base)
        xt = np.empty((T, INPUT + 1, BL), bf16)
        xt[:, 0:INPUT, :] = xs[c].transpose(1, 2, 0).astype(bf16)
        xt[:, INPUT, :] = 1.0
        m["xT"] = xt
        in_maps.append(m)
    return in_maps


_CACHED_NC = None


def kernel(**inputs):
    global _CACHED_NC
    from concourse.bass_utils import run_bass_kernel_spmd

    if _CACHED_NC is None:
        _CACHED_NC = build_nc()
    nc = _CACHED_NC
    in_maps = make_in_maps(**inputs)
    res = run_bass_kernel_spmd(nc, in_maps, list(range(NCORES)))
    outs = [res.results[c]["out"].reshape(BL) for c in range(NCORES)]
    return np.concatenate(outs) + np.float32(inputs["bfc"][0])


# revision 6
# speedup vs baseline: 1.0246x; 1.0024x over previous
"""Trainium2 Bass kernel for a 2-layer LSTM (H=64) + FC head.

Problem: x [4096, 168, 19] f32 -> out [4096] f32
  h1 = LSTM0(x); h2 = LSTM1(h1); out = h2[:, -1, :] @ Wfc.T + bfc

Data-parallel over 8 NeuronCores (512 batch rows each). Per core the
batch is split into CH=3 independent chains whose serial recurrences
interleave on the engines. Layer 0 at time w and layer 1 at time w-1
share one "wave" so every op uses all 128 partitions (p0:64 = layer0,
p64:128 = layer1).

All-tanh formulation: the sigmoid gates' weight rows are pre-halved on
the host so sigma(z) = tanh(z/2)*0.5+0.5; one ACT instruction then
applies tanh over the whole 4-bank z tile [G|I|F|O], and the sigmoid
affine runs on DVE (tensor_scalar, bf16 4x mode).

Software pipelining: tanh(c') and h = sigma_o * tanh(c') for wave w are
emitted at the TOP of wave w+1, so the ACT stream per wave is
[stc_c, T1_c] per chain and the c'->tanh->h->matmul round trip overlaps
the other chains' gate work.

Per wave per chain: PE: 8 bf16 matmuls (N=CB); ACT: stc tanh [128,CB]
+ T1 tanh [128,4CB]; DVE: sigma ts [128,3CB] bf16 4x + u mul bf16 2x;
Pool: h mul, v mul, c' add (no access bubble on Pool). The c'-path ops
are column-halved so the second half pipelines behind the first.
Chain sizes (128,232,152) tessellate the in-order ACT stream
[stc_A,stc_B,stc_C,T1_A,T1_B,T1_C] to ~49ns idle per 3293ns wave
(ACT busy floor 3244ns).
"""

import numpy as np

HIDDEN = 64
INPUT = 19
B = 4096
T = 168
NCORES = 8
BL = B // NCORES   # 512 per core
CBS = (136, 232, 144)
CH = len(CBS)
COFF = (0, 136, 368)
H4 = 4 * HIDDEN    # 256

# torch gate order rows: i(0:64) f(64:128) g(128:192) o(192:256)
# our bank (column-block) order: G, I, F, O
GATE_PERM = np.concatenate([
    np.arange(128, 192),  # g
    np.arange(0, 64),     # i
    np.arange(64, 128),   # f
    np.arange(192, 256),  # o
])
# tanh(z/2) trick for sigmoid gates: halve I, F, O rows (banks 1..3)
BANK_SCALE = np.array([1.0, 0.5, 0.5, 0.5], np.float32)


def build_nc(steps=T):
    import concourse.bacc as bacc
    import concourse.tile as tile
    from concourse import mybir

    F32 = mybir.dt.float32
    BF16 = mybir.dt.bfloat16
    MMD = BF16  # matmul operand dtype (1 cycle/row at any N)
    AF = mybir.ActivationFunctionType
    OP = mybir.AluOpType

    nc = bacc.Bacc("TRN2", target_bir_lowering=False, debug=False,
                   num_devices=NCORES)

    xT = nc.dram_tensor("xT", [T, INPUT + 1, BL], MMD, kind="ExternalInput")
    w0x_d = nc.dram_tensor("w0x", [INPUT + 1, 512], MMD, kind="ExternalInput")
    whbig_d = nc.dram_tensor("whbig", [128, 512], MMD, kind="ExternalInput")
    wfc_d = nc.dram_tensor("wfc", [128, 1], MMD, kind="ExternalInput")
    out = nc.dram_tensor("out", [1, BL], F32, kind="ExternalOutput")

    with tile.TileContext(nc) as tc:
        with (
            tc.tile_pool(name="const", bufs=1) as const,
            tc.tile_pool(name="state", bufs=1) as state,
            tc.tile_pool(name="work", bufs=3) as work,
            tc.tile_pool(name="xin", bufs=5) as xin,
            tc.tile_pool(name="zpool", bufs=CH, space="PSUM") as zpool,
        ):
            w0x = const.tile([INPUT + 1, 4, 128], MMD, tag="w0x", name="w0x")
            whbig = const.tile([128, 4, 128], MMD, tag="wh", name="whbig")
            wfc = const.tile([128, 1], MMD, tag="wfc", name="wfc")
            nc.sync.dma_start(w0x, w0x_d[:])
            nc.scalar.dma_start(whbig, whbig_d[:])
            nc.scalar.dma_start(wfc, wfc_d[:])

            # per-chain state: C = cell (f32), hm = [h0; h1] (matmul rhs)
            C = [[state.tile([128, CBS[c]], F32, tag=f"C{c}{p}",
                             name=f"C{c}{p}") for p in (0, 1)]
                 for c in range(CH)]
            hm = [[state.tile([128, CBS[c]], MMD, tag=f"hm{c}{p}",
                              name=f"hm{c}{p}") for p in (0, 1)]
                  for c in range(CH)]
            for c in range(CH):
                nc.vector.memset(C[c][0], 0.0)
                nc.gpsimd.memset(hm[c][0], 0.0)

            nwaves = steps + 1
            so_prev = [None] * CH
            stc_bufs = [None] * CH

            def stage2(c, cur, partial=False):
                # tanh(c'@w-1) and h@w-1 = sigma_o * tanh(c'); writes hm[cur]
                CB = CBS[c]
                stc = work.tile([128, CB], BF16, tag=f"stc{c}",
                                name=f"stc{c}")
                lo = 64 if partial else 0
                nc.scalar.activation(stc[lo:128], C[c][cur][lo:128], AF.Tanh)
                h2 = CB // 2
                for s in (slice(0, h2), slice(h2, CB)):
                    nc.gpsimd.tensor_mul(hm[c][cur][lo:128, s],
                                         so_prev[c][lo:128, s],
                                         stc[lo:128, s])

            def stage1(c, w, cur, nxt):
                CB = CBS[c]
                cs = slice(COFF[c], COFF[c] + CB)
                xt = xin.tile([INPUT + 1, CB], MMD, tag=f"x{c}", name=f"x{c}")
                nc.sync.dma_start(xt, xT[w % T, :, cs])

                # pad bank slots to 256-f32 alignment so each matmul
                # output sits inside one 2KB PSUM bank
                z = zpool.tile([128, 4, CB], F32, tag="z", name=f"z{c}",
                               padded_shape=[128, 4, 256])
                # per gate-bank: mm1 = x-part + biases (K=20, ones-row
                # trick carries b0|b1); mm2 adds both layers' h-parts
                # (K=128) from hm = [h0; h1].
                m2 = CB // 2
                for b in range(4):
                    nc.tensor.matmul(z[:, b, :], w0x[:, b, :], xt[:],
                                     start=True, stop=False,
                                     skip_group_check=True)
                    nc.tensor.matmul(z[:, b, 0:m2], whbig[:, b, :],
                                     hm[c][cur][:, 0:m2], start=False,
                                     stop=True, skip_group_check=True)
                    nc.tensor.matmul(z[:, b, m2:CB], whbig[:, b, :],
                                     hm[c][cur][:, m2:CB], start=False,
                                     stop=True, skip_group_check=True)

                t = work.tile([128, 4, CB], BF16, tag=f"t{c}", name=f"t{c}")
                nc.scalar.activation(t[:, :, :], z[:, :, :], AF.Tanh)
                sig = work.tile([128, 3, CB], BF16, tag=f"sig{c}",
                                name=f"sig{c}")
                u = work.tile([128, CB], BF16, tag=f"u{c}", name=f"u{c}")
                v = work.tile([128, CB], F32, tag=f"v{c}", name=f"v{c}")
                # column-halved c'-path: the second half of ts/u/v/c'
                # pipelines behind the first, shortening c' readiness
                hh = CB // 2
                for s in (slice(0, hh), slice(hh, CB)):
                    nc.vector.tensor_scalar(sig[:, :, s], t[:, 1:4, s],
                                            0.5, 0.5, OP.mult, OP.add)
                    nc.vector.tensor_mul(u[:, s], sig[:, 0, s], t[:, 0, s])
                    nc.gpsimd.tensor_mul(v[:, s], sig[:, 1, s],
                                         C[c][cur][:, s])
                    nc.gpsimd.tensor_add(C[c][nxt][:, s], u[:, s], v[:, s])
                so_prev[c] = sig[:, 2, :]

            for w in range(nwaves):
                cur, nxt = w % 2, (w + 1) % 2
                if w > 0:
                    for c in range(CH):
                        stage2(c, cur)
                for c in range(CH):
                    stage1(c, w, cur, nxt)
                if w == 0:
                    # wave 0's layer-1 half ran on garbage; zero its cell
                    # state; h@0 layer-1 = sigma_o*tanh(0) = 0 follows.
                    for c in range(CH):
                        nc.gpsimd.memset(C[c][nxt][64:128], 0.0)

            # --- tail: flush h@last (only layer-1 half matters) + FC ---
            # pfc -> o_sb copies ride the idle DVE so they overlap the
            # tail stc stream on ACT; one DMA ships the assembled row.
            cur_t = nwaves % 2
            o_sb = work.tile([1, BL], F32, tag="osb", name="o_sb")
            for c in range(CH):
                stage2(c, cur_t, partial=True)
            for c in range(CH):
                pfc = zpool.tile([1, CBS[c]], F32, tag="z", name=f"pfc{c}")
                nc.tensor.matmul(pfc, wfc, hm[c][cur_t][:],
                                 start=True, stop=True)
                nc.vector.tensor_copy(
                    o_sb[:, COFF[c]:COFF[c] + CBS[c]], pfc)
            nc.sync.dma_start(out[:], o_sb)

    nc.compile()
    return nc


def make_in_maps(x, Wih0, Whh0, bih0, bhh0, Wih1, Whh1, bih1, bhh1, Wfc, bfc):
    """Shard + pre-transpose/concat inputs for the 8 cores."""
    p = GATE_PERM
    s = np.repeat(BANK_SCALE, HIDDEN)  # [256] per permuted gate row
    b0 = (bih0 + bhh0)[p].astype(np.float32) * s
    b1 = (bih1 + bhh1)[p].astype(np.float32) * s
    W0i = Wih0[p] * s[:, None]
    W0h = Whh0[p] * s[:, None]
    W1i = Wih1[p] * s[:, None]
    W1h = Whh1[p] * s[:, None]
    # w0x [20, 4, 128]: rows = [x features (19); ones]. Left cols =
    # [Wih0; b0] per gate, right cols = b1 on the ones row.
    # whbig [128, 4, 128]: left cols = [Whh0; 0], right cols =
    # [Wih1; Whh1] -- one K=128 matmul vs hm covers both layers.
    w0x = np.zeros((INPUT + 1, 4, 128), np.float32)
    whbig = np.zeros((128, 4, 128), np.float32)
    for b in range(4):
        w0x[0:INPUT, b, 0:64] = W0i.T[:, b * 64:(b + 1) * 64]
        w0x[INPUT, b, 0:64] = b0[b * 64:(b + 1) * 64]
        w0x[INPUT, b, 64:128] = b1[b * 64:(b + 1) * 64]
        whbig[0:64, b, 0:64] = W0h.T[:, b * 64:(b + 1) * 64]
        whbig[0:64, b, 64:128] = W1i.T[:, b * 64:(b + 1) * 64]
        whbig[64:128, b, 64:128] = W1h.T[:, b * 64:(b + 1) * 64]
    wfcbig = np.zeros((128, 1), np.float32)
    wfcbig[64:128, 0] = Wfc.reshape(HIDDEN)
    import ml_dtypes
    bf16 = ml_dtypes.bfloat16
    base = {
        "w0x": np.ascontiguousarray(w0x.reshape(INPUT + 1, 512)).astype(bf16),
        "whbig": np.ascontiguousarray(whbig.reshape(128, 512)).astype(bf16),
        "wfc": wfcbig.astype(bf16),
    }
    xs = x.reshape(NCORES, BL, T, INPUT)
    in_maps = []
    for c in range(NCORES):
        m = dict(base)
        xt = np.empty((T, INPUT + 1, BL), bf16)
        xt[:, 0:INPUT, :] = xs[c].transpose(1, 2, 0).astype(bf16)
        xt[:, INPUT, :] = 1.0
        m["xT"] = xt
        in_maps.append(m)
    return in_maps


_CACHED_NC = None


def kernel(**inputs):
    global _CACHED_NC
    from concourse.bass_utils import run_bass_kernel_spmd

    if _CACHED_NC is None:
        _CACHED_NC = build_nc()
    nc = _CACHED_NC
    in_maps = make_in_maps(**inputs)
    res = run_bass_kernel_spmd(nc, in_maps, list(range(NCORES)))
    outs = [res.results[c]["out"].reshape(BL) for c in range(NCORES)]
    return np.concatenate(outs) + np.float32(inputs["bfc"][0])
